# revision 1
# baseline (speedup 1.0000x reference)
"""CoordAtt3D (N,C,D,H,W = 4,64,64,64,64) on 8 Trainium2 NeuronCores.

Sharding: each core owns (sample n = i//2, channel half cs = (i%2)*32), i.e.
a [32, 64, 64, 64] slab of x (32 MiB).  All three adaptive pools are
per-channel, hence fully local.  The only cross-core step is the conv1
channel contraction (8x64), a 6 KB exchange between core pairs.

Per-core program (v2 — fp16 stash, single HBM pass):
  Phase P: stream x in 16 chunks of [128, 4096] fp32 (partition
           p = dl*32 + c, d = 4k+dl).  Per chunk: Act converts to an
           fp16 stash tile (kept resident — 16 MiB total), DVE row-sums
           over w, PE accumulates the d-sum S in PSUM from the fp16 copy
           (1 cycle/row vs 4 for fp32).
  Mid:     conv1 matmul -> pair exchange (AllReduce or AllGather+add) ->
           BN+hardswish -> three tiny convs + sigmoid -> attention
           vectors; M4[p,(h,w)] = ah[c,h]*aw[c,w] built on 32 partitions
           (DVE) and replicated to 128 via a PE matmul into PSUM.
  Phase M: out_chunk = (stash16 * ad[p]) * M4 in one fused DVE op per
           chunk (no HBM re-read), stream out.
"""

import os
import sys

if "/opt/trn_rl_repo" not in sys.path:
    sys.path.insert(0, "/opt/trn_rl_repo")

import numpy as np

_DEBUG_STAGE = os.environ.get("KERNEL_DEBUG_STAGE", "full")

N, C, D, H, W = 4, 64, 64, 64, 64
MIP = 8
BN_EPS = 1e-5
NCORES = 8
CL = C // 2          # 32 channels per core
DL = 4               # d-values per chunk
NK = D // DL         # 16 chunks
FREE = H * W         # 4096
P = 128

_PROGS = {}


def _build_program(reps=1, stage=None):
    if stage is None:
        stage = _DEBUG_STAGE
    if (stage, reps) in _PROGS:
        return _PROGS[(stage, reps)]

    import concourse.bacc as bacc
    import concourse.mybir as mybir
    import concourse.tile as tile

    fp32 = mybir.dt.float32
    fp16 = mybir.dt.float16
    AX = mybir.AxisListType
    OP = mybir.AluOpType
    AF = mybir.ActivationFunctionType

    nc = bacc.Bacc("TRN2", target_bir_lowering=False, debug=False,
                   num_devices=NCORES)

    xc = nc.dram_tensor("xc", [CL, D, H, W], fp32, kind="ExternalInput")
    w1t = nc.dram_tensor("w1t", [CL, MIP], fp32, kind="ExternalInput")
    wdt = nc.dram_tensor("wdt", [MIP, CL], fp32, kind="ExternalInput")
    wht = nc.dram_tensor("wht", [MIP, CL], fp32, kind="ExternalInput")
    wwt = nc.dram_tensor("wwt", [MIP, CL], fp32, kind="ExternalInput")
    bdt = nc.dram_tensor("bd", [CL, 1], fp32, kind="ExternalInput")
    bht = nc.dram_tensor("bh", [CL, 1], fp32, kind="ExternalInput")
    bwt = nc.dram_tensor("bw", [CL, 1], fp32, kind="ExternalInput")
    s1t = nc.dram_tensor("s1", [MIP, 1], fp32, kind="ExternalInput")
    t1t = nc.dram_tensor("t1", [MIP, 1], fp32, kind="ExternalInput")
    onest = nc.dram_tensor("ones16", [P, CL], fp16, kind="ExternalInput")
    repht = nc.dram_tensor("rep128", [CL, P], fp16, kind="ExternalInput")
    outt = nc.dram_tensor("out", [CL, D, H, W], fp32, kind="ExternalOutput")
    dbgt = nc.dram_tensor("dbg", [P, 256], fp32, kind="ExternalOutput")

    # chunk k covers d = k*DL + dl; SBUF partition p = dl*32 + c
    # DRAM view [k][dl, c, (h w)] pairs with a plain [128, 4096] SBUF tile
    xv = xc[:].rearrange("c (k dl) h w -> k dl c (h w)", dl=DL)
    ov = outt[:].rearrange("c (k dl) h w -> k dl c (h w)", dl=DL)

    with tile.TileContext(nc) as tc:
        with tc.tile_pool(name="const", bufs=1) as cp, \
             tc.tile_pool(name="acc", bufs=1) as accp, \
             tc.tile_pool(name="small", bufs=3) as smp, \
             tc.tile_pool(name="xio", bufs=3) as xio, \
             tc.tile_pool(name="psum", bufs=1, space="PSUM") as pp, \
             tc.tile_pool(name="dram", bufs=1, space="DRAM") as dp:

            if stage == "copy":
                for _ in range(reps):
                  for k in range(NK):
                    t = xio.tile([P, FREE], fp32, tag="io")
                    for dl in range(DL):
                        nc.sync.dma_start(t[dl * CL:(dl + 1) * CL, :],
                                          xv[k][dl])
                    for dl in range(DL):
                        nc.scalar.dma_start(ov[k][dl],
                                            t[dl * CL:(dl + 1) * CL, :])
            else:
                if stage == "att0":
                    # keep has_collectives (and the NEFF entry barrier)
                    # comparable with the collective stages
                    din = dp.tile([1, 4], fp32)
                    dout = dp.tile([2, 4], fp32)
                    nc.gpsimd.collective_compute(
                        "AllGather", OP.bypass,
                        replica_groups=[[0, 1], [2, 3], [4, 5], [6, 7]],
                        ins=[din[:].opt()], outs=[dout[:].opt()])
                for _ in range(reps):
                    _body(nc, tc, stage, cp, accp, smp, xio, pp, dp,
                          fp32, fp16, AX, OP, AF,
                          xc, w1t, wdt, wht, wwt, bdt, bht, bwt, s1t, t1t,
                          onest, repht, outt, dbgt, xv, ov)

    nc.compile()
    _PROGS[(stage, reps)] = nc
    return nc


def _body(nc, tc, stage, cp, accp, smp, xio, pp, dp, fp32, fp16, AX, OP, AF,
          xc, w1t, wdt, wht, wwt, bdt, bht, bwt, s1t, t1t, onest, repht,
          outt, dbgt, xv, ov):
    w1s = cp.tile([CL, MIP], fp32)
    nc.sync.dma_start(w1s[:], w1t[:])
    wds = cp.tile([MIP, CL], fp32)
    nc.sync.dma_start(wds[:], wdt[:])
    whs = cp.tile([MIP, CL], fp32)
    nc.sync.dma_start(whs[:], wht[:])
    wws = cp.tile([MIP, CL], fp32)
    nc.sync.dma_start(wws[:], wwt[:])
    bds = cp.tile([CL, 1], fp32)
    nc.sync.dma_start(bds[:], bdt[:])
    bhs = cp.tile([CL, 1], fp32)
    nc.sync.dma_start(bhs[:], bht[:])
    bws = cp.tile([CL, 1], fp32)
    nc.sync.dma_start(bws[:], bwt[:])
    s1s = cp.tile([MIP, 1], fp32)
    nc.sync.dma_start(s1s[:], s1t[:])
    t1s = cp.tile([MIP, 1], fp32)
    nc.sync.dma_start(t1s[:], t1t[:])
    ones = cp.tile([P, CL], fp16)
    nc.sync.dma_start(ones[:], onest[:])
    reph = cp.tile([CL, P], fp16)
    nc.sync.dma_start(reph[:], repht[:])

    xd_all = accp.tile([P, NK], fp32)
    rw_all = accp.tile([P, NK * H], fp32)
    pools = accp.tile([CL, 3 * 64], fp32)
    NB = FREE // 512  # 8 PSUM banks
    S = pp.tile([CL, FREE], fp32, tag="ps")

    # ---------------- Phase P: pooled sums + fp16 stash ----------------
    # DVE: rw_all[p, k*64+h] = sum_w chunk_k  (per-(c,d) row sums -> xd)
    # Act: stash16[k] = fp16(chunk_k)         (resident for phase M)
    # PE:  S[c, (h,w)] += sum_dl stash16[k]   (sum over d -> xh, xw)
    stash = [accp.tile([P, FREE], fp16, name=f"stash{k}")
             for k in range(NK)]
    for k in range(NK):
        t = xio.tile([P, FREE], fp32, tag="io")
        for dl in range(DL):
            nc.sync.dma_start(t[dl * CL:(dl + 1) * CL, :], xv[k][dl])
        nc.scalar.activation(stash[k][:], t[:], AF.Identity)
        nc.vector.tensor_reduce(
            rw_all[:, k * H:(k + 1) * H],
            t[:].rearrange("p (h w) -> p h w", w=W),
            axis=AX.X, op=OP.add)
        for b in range(NB):
            nc.tensor.matmul(S[:, b * 512:(b + 1) * 512], ones[:],
                             stash[k][:, b * 512:(b + 1) * 512],
                             start=(k == 0), stop=(k == NK - 1))

    # ---------------- fold to pools [32, 192] ----------------
    # d-section columns are permuted: col j = dl*16 + k <-> d = 4k+dl
    nc.vector.tensor_reduce(
        xd_all[:], rw_all[:].rearrange("p (k h) -> p k h", h=H),
        axis=AX.X, op=OP.add)
    for dl in range(DL):
        nc.vector.tensor_copy(
            pools[:, dl * NK:(dl + 1) * NK],
            xd_all[dl * CL:(dl + 1) * CL, :])
    nc.vector.tensor_reduce(
        pools[:, 64:128], S[:].rearrange("c (h w) -> c h w", w=W),
        axis=AX.X, op=OP.add)
    nc.vector.tensor_reduce(
        pools[:, 128:192], S[:].rearrange("c (h w) -> c w h", w=W),
        axis=AX.X, op=OP.add)

    if stage == "pool":
        nc.sync.dma_start(dbgt[0:CL, 0:192], pools[:])
        return

    # ---------------- conv1 + pair exchange ----------------
    py1 = pp.tile([MIP, 192], fp32, tag="ps")
    nc.tensor.matmul(py1[:], w1s[:], pools[:])
    y1 = smp.tile([MIP, 192], fp32, tag="y1")
    nc.vector.tensor_copy(y1[:], py1[:])

    y1r = smp.tile([MIP, 192], fp32, tag="y1r")
    if stage == "att0":
        # timing-only: skip the exchange (results are wrong)
        nc.vector.tensor_copy(y1r[:], y1[:])
    elif stage in ("full", "ccag", "ccag2"):
        # pair AllGather + local add (order-symmetric)
        cin = dp.tile([MIP, 192], fp32)
        cout = dp.tile([2, MIP, 192], fp32)
        nc.sync.dma_start(cin[:], y1[:])
        nc.gpsimd.collective_compute(
            "AllGather", OP.bypass,
            replica_groups=[[0, 1], [2, 3], [4, 5], [6, 7]],
            ins=[cin[:].opt()], outs=[cout[:].opt()])
        if stage == "ccag2":
            cout2 = dp.tile([2, MIP, 192], fp32)
            nc.gpsimd.collective_compute(
                "AllGather", OP.bypass,
                replica_groups=[[0, 1], [2, 3], [4, 5], [6, 7]],
                ins=[cin[:].opt()], outs=[cout2[:].opt()])
        yg0 = smp.tile([MIP, 192], fp32, tag="yg0")
        nc.sync.dma_start(yg0[:], cout[0])
        yg1 = smp.tile([MIP, 192], fp32, tag="yg1")
        nc.sync.dma_start(yg1[:], cout[1])
        nc.vector.tensor_tensor(y1r[:], yg0[:], yg1[:], op=OP.add)
    elif stage == "ccag16":
        # pair AllGather of fp16 y1 (3 KB) + local add
        y116 = smp.tile([MIP, 192], fp16, tag="y116")
        nc.scalar.activation(y116[:], y1[:], AF.Identity)
        cin = dp.tile([MIP, 192], fp16)
        cout = dp.tile([2, MIP, 192], fp16)
        nc.sync.dma_start(cin[:], y116[:])
        nc.gpsimd.collective_compute(
            "AllGather", OP.bypass,
            replica_groups=[[0, 1], [2, 3], [4, 5], [6, 7]],
            ins=[cin[:].opt()], outs=[cout[:].opt()])
        yg0 = smp.tile([MIP, 192], fp16, tag="yg0")
        nc.sync.dma_start(yg0[:], cout[0])
        yg1 = smp.tile([MIP, 192], fp16, tag="yg1")
        nc.sync.dma_start(yg1[:], cout[1])
        nc.vector.tensor_tensor(y1r[:], yg0[:], yg1[:], op=OP.add)
    else:
        cin = dp.tile([MIP, 192], fp32)
        cout = dp.tile([MIP, 192], fp32)
        nc.sync.dma_start(cin[:], y1[:])
        if stage == "nocc":
            nc.sync.dma_start(cout[:], cin[:])
        else:
            nc.gpsimd.collective_compute(
                "AllReduce", OP.add,
                replica_groups=[[0, 1], [2, 3], [4, 5], [6, 7]],
                ins=[cin[:].opt()], outs=[cout[:].opt()])
        nc.sync.dma_start(y1r[:], cout[:])

    # BN (folded scale/shift incl. /4096) + hardswish
    ybn = smp.tile([MIP, 192], fp32, tag="ybn")
    nc.scalar.activation(ybn[:], y1r[:], AF.Identity,
                         bias=t1s[:], scale=s1s[:])
    hs = smp.tile([MIP, 192], fp32, tag="hs")
    half_b = cp.tile([MIP, 1], fp32)
    nc.vector.memset(half_b[:], 0.5)
    # relu(v/6 + 0.5) == relu6(v+3)/6 before the min-with-1 clamp
    nc.scalar.activation(hs[:], ybn[:], AF.Relu,
                         bias=half_b[:], scale=1.0 / 6.0)
    nc.vector.tensor_scalar_min(hs[:], hs[:], 1.0)
    yact = smp.tile([MIP, 192], fp32, tag="yact")
    nc.vector.tensor_tensor(yact[:], ybn[:], hs[:], op=OP.mult)

    # ---------------- three tiny convs + sigmoid ----------------
    att = accp.tile([CL, 192], fp32)  # [ad_perm | ah | aw]
    for ws, bs, off in ((whs, bhs, 64), (wws, bws, 128), (wds, bds, 0)):
        pa = pp.tile([CL, 64], fp32, tag="ps")
        nc.tensor.matmul(pa[:], ws[:], yact[:, off:off + 64])
        nc.scalar.activation(att[:, off:off + 64], pa[:], AF.Sigmoid,
                             bias=bs[:], scale=1.0)

    if stage == "mid":
        nc.sync.dma_start(dbgt[0:CL, 0:192], att[:])
        return

    # M4_32[c, (h,w)] = ah[c,h] * aw[c,w] in fp16 on 32 partitions,
    # then PE-replicate to all 128 partitions (PSUM), p -> c = p % 32
    m32 = accp.tile([CL, FREE], fp16)
    nc.vector.tensor_tensor(
        m32[:].rearrange("c (h w) -> c h w", w=W),
        att[:, 64:128].unsqueeze(2).broadcast_to([CL, H, W]),
        att[:, 128:192].unsqueeze(1).broadcast_to([CL, H, W]),
        op=OP.mult)
    M4 = pp.tile([P, FREE], fp32, tag="ps")
    for b in range(NB):
        nc.tensor.matmul(M4[:, b * 512:(b + 1) * 512], reph[:],
                         m32[:, b * 512:(b + 1) * 512])

    # ad per (chunk, partition): ad_pm[dl*32+c, k] = ad[c, 4k+dl]
    ad_pm = accp.tile([P, NK], fp32)
    for dl in range(DL):
        nc.vector.tensor_copy(ad_pm[dl * CL:(dl + 1) * CL, :],
                              att[:, dl * NK:(dl + 1) * NK])

    # ---------------- Phase M: fused broadcast multiply ----------
    for k in range(NK):
        tout = xio.tile([P, FREE], fp32, tag="io")
        nc.vector.scalar_tensor_tensor(
            tout[:], stash[k][:], ad_pm[:, k:k + 1], M4[:],
            op0=OP.mult, op1=OP.mult)
        for dl in range(DL):
            nc.scalar.dma_start(ov[k][dl], tout[dl * CL:(dl + 1) * CL, :])


def _host_inputs(x, conv1_w, conv1_b, bn_gamma, bn_beta, bn_mean, bn_var,
                 convd_w, convd_b, convh_w, convh_b, convw_w, convw_b):
    scale = bn_gamma / np.sqrt(bn_var + BN_EPS)
    s1 = (scale / float(FREE)).astype(np.float32).reshape(MIP, 1)
    t1 = ((conv1_b - bn_mean) * scale + bn_beta).astype(np.float32) \
        .reshape(MIP, 1)
    ones16 = (np.arange(P)[:, None] % CL ==
              np.arange(CL)[None, :]).astype(np.float16)
    rep128 = (np.arange(P)[None, :] % CL ==
              np.arange(CL)[:, None]).astype(np.float16)
    in_maps = []
    for i in range(NCORES):
        n, half = i // 2, i % 2
        cs = half * CL
        in_maps.append({
            "xc": np.ascontiguousarray(x[n, cs:cs + CL]),
            "w1t": np.ascontiguousarray(conv1_w[:, cs:cs + CL].T),
            "wdt": np.ascontiguousarray(convd_w[cs:cs + CL, :].T),
            "wht": np.ascontiguousarray(convh_w[cs:cs + CL, :].T),
            "wwt": np.ascontiguousarray(convw_w[cs:cs + CL, :].T),
            "bd": np.ascontiguousarray(convd_b[cs:cs + CL].reshape(CL, 1)),
            "bh": np.ascontiguousarray(convh_b[cs:cs + CL].reshape(CL, 1)),
            "bw": np.ascontiguousarray(convw_b[cs:cs + CL].reshape(CL, 1)),
            "s1": s1,
            "t1": t1,
            "ones16": ones16,
            "rep128": rep128,
        })
    return in_maps


def _run(in_maps, trace=False):
    from concourse.bass_utils import run_bass_kernel_spmd
    nc = _build_program()
    return run_bass_kernel_spmd(nc, in_maps, list(range(NCORES)),
                                trace=trace)


def kernel(**inputs):
    args = {k: np.asarray(v, dtype=np.float32) for k, v in inputs.items()}
    in_maps = _host_inputs(**args)
    res = _run(in_maps)
    y = np.empty((N, C, D, H, W), dtype=np.float32)
    for i in range(NCORES):
        n, half = i // 2, i % 2
        y[n, half * CL:(half + 1) * CL] = res.results[i]["out"]
    return y

